# revision 11
# baseline (speedup 1.0000x reference)
"""Bass/Trainium2 kernel for nn_Expert_WNO2d (8-expert gated WaveConv2d mixture).

Math: the reference is linear in x. Every expert passes the fine Haar detail
levels (1..3) through unchanged and only channel-mixes the coarsest (level-4)
approximation + detail coefficients. With gate slots s weighting experts
PERM = (0,1,2,3,4,5,4,5), the output collapses to

    y[b] = G[b] * x[b] + rep8( adj[b] )                      (rep8 = 8x8 block broadcast)
    adj[b] = 0.125 * idwt4( sum_e geff[b,e] * (W_e . c4[b]) ) - (G[b]/64) * s8[b]

where s8 = 8x8 block sums of x, c4 = level-4 Haar coefficients (from s8),
G[b] = sum_s lambda[b,s], geff[b,e] = gate mass routed to expert e.

Sharding: data-parallel over batch B=32 across 8 cores (4 samples/core);
the [6,4,C,C,4,4] expert weights are replicated in bf16 (packed host-side
into the exact SBUF image, scaled by 0.0625 to fold the idwt/broadcast
constants). x streams in 1MB sub-tiles with partial block-sum reduces;
weights stream per-band so matmuls start before the full load; the final
fused pass streams per sub-tile (DVE + GpSimd) so y DMA-out overlaps.
"""

import numpy as np

import concourse.bacc as bacc
import concourse.mybir as mybir
import concourse.tile as tile

N_CORES = 8
B, C, S = 32, 64, 64
BL = B // N_CORES          # samples per core = 4
NE = 6                     # live experts
NCH = 4                    # x sub-tiles per row-tile
f32 = mybir.dt.float32
bf16 = mybir.dt.bfloat16
ALU = mybir.AluOpType


def _build_nc():
    nc = bacc.Bacc()
    xw = nc.declare_dram_parameter("xw", [2, 128, 4096], f32, isOutput=False)
    wt = nc.declare_dram_parameter("wt", [4, 128, 3072], bf16, isOutput=False)
    gt = nc.declare_dram_parameter("gt", [2, 128, 8], f32, isOutput=False)
    yw = nc.declare_dram_parameter("yw", [2, 128, 4096], f32, isOutput=True)

    with tile.TileContext(nc) as tc:
        with (
            tc.tile_pool(name="xp", bufs=8) as xp,
            tc.tile_pool(name="yp", bufs=8) as yp,
            tc.tile_pool(name="wp", bufs=4) as wp,
            tc.tile_pool(name="sp", bufs=2) as sp,
            tc.tile_pool(name="cp", bufs=3) as cp,
            tc.tile_pool(name="tp", bufs=8) as ttp,
            tc.tile_pool(name="ps", bufs=4, space="PSUM") as psp,
        ):
            gt_s, xs = [], [[], []]
            for rt in range(2):
                g = sp.tile([128, 8], f32, tag="gt", name=f"g{rt}")
                nc.sync.dma_start(out=g[:, :], in_=gt[rt, :, :])
                gt_s.append(g)

            wt_b = []
            for band in range(4):
                w = wp.tile([128, 3072], bf16, tag="wt", name=f"w{band}")
                wt_b.append(w)

            # interleave x sub-tile and weight-band DMA issue; x leads
            order = [("x", 0, 0), ("x", 0, 1), ("x", 0, 2), ("x", 0, 3),
                     ("x", 1, 0), ("x", 1, 1), ("x", 1, 2), ("x", 1, 3),
                     ("w", 0, 0), ("w", 1, 0), ("w", 2, 0), ("w", 3, 0)]
            for kind, a, c in order:
                if kind == "x":
                    xt = xp.tile([128, 1024], f32, tag="xs", name=f"x{a}{c}")
                    nc.sync.dma_start(out=xt[:, :], in_=xw[a, :, 1024 * c:1024 * (c + 1)])
                    xs[a].append(xt)
                else:
                    nc.sync.dma_start(out=wt_b[a][:, :], in_=wt[a, :, :])

            cc = [cp.tile([128, 256], bf16, tag="cc", name=f"cc{i}") for i in range(3)]
            coef, s8 = [], []
            for rt in range(2):
                # 8x8 block sums, streamed per sub-tile (w-dir), then h-dir
                r1 = sp.tile([128, 512], f32, tag="r1", name=f"r1{rt}")
                for c in range(NCH):
                    nc.vector.tensor_reduce(
                        out=r1[:, 128 * c:128 * (c + 1)].rearrange("p (h v) -> p h v", h=16),
                        in_=xs[rt][c][:, :].rearrange("p (h v w) -> p h v w", h=16, v=8, w=8),
                        axis=mybir.AxisListType.X, op=ALU.add,
                    )
                s8t = sp.tile([128, 64], f32, tag="s8", name=f"s8{rt}")
                nc.vector.tensor_reduce(
                    out=s8t[:, :].rearrange("p (u v) -> p u v", u=8),
                    in_=r1[:, :].rearrange("p (u dh v) -> p u v dh", u=8, dh=8, v=8),
                    axis=mybir.AxisListType.X, op=ALU.add,
                )
                s8.append(s8t)

                # level-4 Haar analysis on 0.0625*s8 (scale folds ll3 + one dwt level)
                sc = sp.tile([128, 64], f32, tag="sc", name=f"sc{rt}")
                nc.vector.tensor_scalar(out=sc[:, :], in0=s8t[:, :],
                                        scalar1=0.0625, scalar2=None, op0=ALU.mult)
                # merged quad combines: {t1,t2} = even+odd, {t3,t4} = even-odd
                # even = {a00,a10}: offsets {0,8}; odd = {a01,a11}: offsets {1,9}
                ev = sc[:, 0:64].rearrange("p (x i y j) -> p i j x y",
                                           x=4, i=2, y=4, j=2)[:, :, 0]
                od = sc[:, 0:64].rearrange("p (x i y j) -> p i j x y",
                                           x=4, i=2, y=4, j=2)[:, :, 1]
                tt = ttp.tile([128, 64], f32, tag="tt", name=f"tt{rt}")
                t2v = lambda o: tt[:, 32 * o:32 * (o + 1)].rearrange(
                    "p (g x y) -> p g x y", g=2, x=4, y=4)
                nc.vector.tensor_add(t2v(0), ev, od)   # t1(a00+a01), t2(a10+a11)
                nc.vector.tensor_sub(t2v(1), ev, od)   # t3, t4
                cf = sp.tile([128, 64], f32, tag="coef", name=f"cf{rt}")
                pick = lambda t, o: t[:, :].rearrange(
                    "p (g h m) -> p h g m", g=2, h=2, m=16)[:, o]
                nc.vector.tensor_add(pick(cf, 0), pick(tt, 0), pick(tt, 1))  # ll, hl
                nc.vector.tensor_sub(pick(cf, 1), pick(tt, 0), pick(tt, 1))  # lh, hh
                coef.append(cf)

                # gate-scaled channel-transposed coefficients: cc[ch][el*64+i, b*64+bm]
                for bh in range(2):
                    b = rt * 2 + bh
                    for e in range(NE):
                        ch, el = e // 2, e % 2
                        nc.vector.tensor_tensor(
                            out=cc[ch][el * 64:(el + 1) * 64, b * 64:(b + 1) * 64],
                            in0=cf[bh * 64:(bh + 1) * 64, :],
                            in1=gt_s[rt][bh * 64:(bh + 1) * 64, 1 + e:2 + e]
                                .broadcast_to([64, 64]),
                            op=ALU.mult,
                        )

            # ScalarE prescale y = G*x per sub-tile (overlaps the matmul phase)
            ys_all = [[], []]
            for rt in range(2):
                for c in range(NCH):
                    ys = yp.tile([128, 1024], f32, tag="ys", name=f"y{rt}{c}")
                    nc.scalar.activation(
                        out=ys[:, :], in_=xs[rt][c][:, :],
                        func=mybir.ActivationFunctionType.Copy,
                        bias=0.0, scale=gt_s[rt][:, 0:1],
                    )
                    ys_all[rt].append(ys)

            # per-mode channel mixing, gate-combined via K=(e,i) accumulation
            pb = [psp.tile([64, 64], f32, tag="pb", name=f"pb{i}") for i in range(4)]
            for band in range(4):
                for mode in range(16):
                    for ch in range(3):
                        nc.tensor.matmul(
                            out=pb[band][:, mode * 4:(mode + 1) * 4],
                            lhsT=wt_b[band][:, (mode * 3 + ch) * 64:(mode * 3 + ch + 1) * 64],
                            rhs=cc[ch][:, band * 16 + mode:band * 16 + mode + 193:64],
                            start=(ch == 0), stop=(ch == 2),
                        )

            # level-4 Haar synthesis (scale folded into weights) scattered per-sample
            sb1 = ttp.tile([64, 64], f32, tag="sb1")
            sb3 = ttp.tile([64, 64], f32, tag="sb3")
            nc.vector.tensor_copy(sb1[:, :], pb[1][:, :])
            nc.vector.tensor_copy(sb3[:, :], pb[3][:, :])
            u1 = ttp.tile([64, 64], f32, tag="u1")
            u2 = ttp.tile([64, 64], f32, tag="u2")
            u3 = ttp.tile([64, 64], f32, tag="u3")
            u4 = ttp.tile([64, 64], f32, tag="u4")
            nc.vector.tensor_add(u1[:, :], pb[0][:, :], sb1[:, :])
            nc.vector.tensor_add(u2[:, :], pb[2][:, :], sb3[:, :])
            nc.vector.tensor_sub(u3[:, :], pb[0][:, :], sb1[:, :])
            nc.vector.tensor_sub(u4[:, :], pb[2][:, :], sb3[:, :])

            adj_hs = []
            for rt in range(2):
                at = sp.tile([128, 64], f32, tag="adjT", name=f"at{rt}")
                for bh in range(2):
                    b = rt * 2 + bh
                    ov = at[bh * 64:(bh + 1) * 64, :].rearrange(
                        "p (x di y dj) -> p di dj x y", x=4, di=2, y=4, dj=2)
                    sv = lambda t: t[:, :].rearrange("p (x y b) -> p b x y", x=4, y=4, b=4)[:, b]
                    nc.vector.tensor_add(ov[:, 0, 0], sv(u1), sv(u2))
                    nc.vector.tensor_sub(ov[:, 0, 1], sv(u1), sv(u2))
                    nc.vector.tensor_add(ov[:, 1, 0], sv(u3), sv(u4))
                    nc.vector.tensor_sub(ov[:, 1, 1], sv(u3), sv(u4))
                # adjF = adjT + (-G/64) * s8   (gt col 7 = -G/64)
                adjF = sp.tile([128, 64], f32, tag="adjF", name=f"af{rt}")
                nc.vector.scalar_tensor_tensor(
                    out=adjF[:, :], in0=s8[rt][:, :], scalar=gt_s[rt][:, 7:8],
                    in1=at[:, :], op0=ALU.mult, op1=ALU.add,
                )
                # expand over h-rep: adj_h[p, u*64 + dh*8 + v] = adjF[p, u*8+v]
                adj_h = sp.tile([128, 512], f32, tag="adjh", name=f"ah{rt}")
                nc.vector.tensor_copy(
                    out=adj_h[:, :].rearrange("p (u dh v) -> p u dh v", u=8, dh=8, v=8),
                    in_=adjF[:, :].rearrange("p (u o v) -> p u o v", u=8, o=1, v=8)
                        .broadcast_to([128, 8, 8, 8]),
                )
                adj_hs.append(adj_h)

            # y += rep8(adjF) into the ScalarE-prescaled tiles, then store.
            # GpSimd (slower per op) takes 3 chunks in parallel with DVE's 5.
            gp_set = {(0, 3), (1, 2), (1, 3)}
            for rt, c in [(0, 3), (1, 2), (1, 3), (0, 0), (0, 1), (0, 2), (1, 0), (1, 1)]:
                eng = nc.gpsimd if (rt, c) in gp_set else nc.vector
                ys = ys_all[rt][c]
                eng.tensor_tensor(
                    out=ys[:, :].rearrange("p (hv w) -> p hv w", w=8),
                    in0=ys[:, :].rearrange("p (hv w) -> p hv w", w=8),
                    in1=adj_hs[rt][:, 128 * c:128 * (c + 1)]
                        .rearrange("p (hv o) -> p hv o", o=1)
                        .broadcast_to([128, 128, 8]),
                    op=ALU.add,
                )
                nc.sync.dma_start(out=yw[rt, :, 1024 * c:1024 * (c + 1)], in_=ys[:, :])
    nc.compile()
    return nc


_NC = None


def _get_nc():
    global _NC
    if _NC is None:
        _NC = _build_nc()
    return _NC


def _pack_weights(WL, WH):
    # Wall[band, e, i, o, x, y]; band 0 = WL, bands 1..3 = WH[:, k-1]
    Wall = np.empty((4, NE, C, C, 4, 4), np.float32)
    Wall[0] = WL[:NE]
    for k in range(3):
        Wall[k + 1] = WH[:NE, k]
    Wall *= 0.0625  # folds idwt 0.5 and rep8 0.125 scales
    # wt[band][el*64+i, ((x*4+y)*3 + ch)*64 + o]
    W6 = Wall.reshape(4, 3, 2, C, C, 4, 4)            # band, ch, el, i, o, x, y
    T = W6.transpose(0, 2, 3, 5, 6, 1, 4)             # band, el, i, x, y, ch, o
    import ml_dtypes
    return np.ascontiguousarray(T.reshape(4, 128, 3072)).astype(ml_dtypes.bfloat16)


def _pack_gates(lambda_):
    lam = lambda_.reshape(B, 8).astype(np.float32)
    G = lam.sum(1)
    geff = lam[:, :6].copy()
    geff[:, 4] += lam[:, 6]
    geff[:, 5] += lam[:, 7]
    gt = np.zeros((B, 8), np.float32)
    gt[:, 0] = G
    gt[:, 1:7] = geff
    gt[:, 7] = -G / 64.0
    return gt


def kernel(x, lambda_, WL, WH):
    from concourse.bass_utils import run_bass_kernel_spmd

    nc = _get_nc()
    wt = _pack_weights(np.asarray(WL, np.float32), np.asarray(WH, np.float32))
    gt = _pack_gates(np.asarray(lambda_, np.float32))
    x = np.ascontiguousarray(np.asarray(x, np.float32))

    in_maps = []
    for k in range(N_CORES):
        xl = x[k * BL:(k + 1) * BL].reshape(2, 128, 4096)
        gl = np.repeat(gt[k * BL:(k + 1) * BL], C, axis=0).reshape(2, 128, 8)
        in_maps.append({"xw": np.ascontiguousarray(xl),
                        "wt": wt,
                        "gt": np.ascontiguousarray(gl)})

    res = run_bass_kernel_spmd(nc, in_maps, list(range(N_CORES)))
    out = np.empty((B, C, S, S), np.float32)
    for k in range(N_CORES):
        out[k * BL:(k + 1) * BL] = res.results[k]["yw"].reshape(BL, C, S, S)
    return out


# revision 12
# speedup vs baseline: 1.0548x; 1.0548x over previous
"""Bass/Trainium2 kernel for nn_Expert_WNO2d (8-expert gated WaveConv2d mixture).

Math: the reference is linear in x. Every expert passes the fine Haar detail
levels (1..3) through unchanged and only channel-mixes the coarsest (level-4)
approximation + detail coefficients. With gate slots s weighting experts
PERM = (0,1,2,3,4,5,4,5), the output collapses to

    y[b] = G[b] * x[b] + rep8( adj[b] )                      (rep8 = 8x8 block broadcast)
    adj[b] = 0.125 * idwt4( sum_e geff[b,e] * (W_e . c4[b]) ) - (G[b]/64) * s8[b]

where s8 = 8x8 block sums of x, c4 = level-4 Haar coefficients (from s8),
G[b] = sum_s lambda[b,s], geff[b,e] = gate mass routed to expert e.

Sharding: data-parallel over batch B=32 across 8 cores (4 samples/core);
the [6,4,C,C,4,4] expert weights are replicated in bf16 (packed host-side
into the exact SBUF image, scaled by 0.0625 to fold the idwt/broadcast
constants). x streams in 1MB sub-tiles with partial block-sum reduces;
weights stream per-band so matmuls start before the full load; the final
fused pass streams per sub-tile (DVE + GpSimd) so y DMA-out overlaps.
"""

import numpy as np

import concourse.bacc as bacc
import concourse.mybir as mybir
import concourse.tile as tile

N_CORES = 8
B, C, S = 32, 64, 64
BL = B // N_CORES          # samples per core = 4
NE = 6                     # live experts
NCH = 4                    # x sub-tiles per row-tile
f32 = mybir.dt.float32
bf16 = mybir.dt.bfloat16
ALU = mybir.AluOpType


def _build_nc():
    nc = bacc.Bacc()
    xw = nc.declare_dram_parameter("xw", [2, 128, 4096], f32, isOutput=False)
    wt = nc.declare_dram_parameter("wt", [4, 128, 3072], bf16, isOutput=False)
    gt = nc.declare_dram_parameter("gt", [2, 128, 8], f32, isOutput=False)
    yw = nc.declare_dram_parameter("yw", [2, 128, 4096], f32, isOutput=True)

    with tile.TileContext(nc) as tc:
        with (
            tc.tile_pool(name="xp", bufs=8) as xp,
            tc.tile_pool(name="yp", bufs=8) as yp,
            tc.tile_pool(name="wp", bufs=4) as wp,
            tc.tile_pool(name="sp", bufs=2) as sp,
            tc.tile_pool(name="cp", bufs=3) as cp,
            tc.tile_pool(name="tp", bufs=8) as ttp,
            tc.tile_pool(name="ps", bufs=4, space="PSUM") as psp,
        ):
            gt_s, xs = [], [[], []]
            for rt in range(2):
                g = sp.tile([128, 8], f32, tag="gt", name=f"g{rt}")
                nc.sync.dma_start(out=g[:, :], in_=gt[rt, :, :])
                gt_s.append(g)

            wt_b = []
            for band in range(4):
                w = wp.tile([128, 3072], bf16, tag="wt", name=f"w{band}")
                wt_b.append(w)

            # interleave x sub-tile and weight-band DMA issue; x leads
            order = [("x", 0, 0), ("x", 0, 1), ("x", 0, 2), ("x", 0, 3),
                     ("x", 1, 0), ("x", 1, 1), ("x", 1, 2), ("x", 1, 3),
                     ("w", 0, 0), ("w", 1, 0), ("w", 2, 0), ("w", 3, 0)]
            for kind, a, c in order:
                if kind == "x":
                    xt = xp.tile([128, 1024], f32, tag="xs", name=f"x{a}{c}")
                    nc.sync.dma_start(out=xt[:, :], in_=xw[a, :, 1024 * c:1024 * (c + 1)])
                    xs[a].append(xt)
                else:
                    nc.sync.dma_start(out=wt_b[a][:, :], in_=wt[a, :, :])

            cc = [cp.tile([128, 256], bf16, tag="cc", name=f"cc{i}") for i in range(3)]
            coef, s8 = [], []
            for rt in range(2):
                # 8x8 block sums, streamed per sub-tile (w-dir), then h-dir
                r1 = sp.tile([128, 512], f32, tag="r1", name=f"r1{rt}")
                for c in range(NCH):
                    nc.vector.tensor_reduce(
                        out=r1[:, 128 * c:128 * (c + 1)].rearrange("p (h v) -> p h v", h=16),
                        in_=xs[rt][c][:, :].rearrange("p (h v w) -> p h v w", h=16, v=8, w=8),
                        axis=mybir.AxisListType.X, op=ALU.add,
                    )
                s8t = sp.tile([128, 64], f32, tag="s8", name=f"s8{rt}")
                nc.vector.tensor_reduce(
                    out=s8t[:, :].rearrange("p (u v) -> p u v", u=8),
                    in_=r1[:, :].rearrange("p (u dh v) -> p u v dh", u=8, dh=8, v=8),
                    axis=mybir.AxisListType.X, op=ALU.add,
                )
                s8.append(s8t)

                # level-4 Haar analysis on 0.0625*s8 (scale folds ll3 + one dwt level)
                sc = sp.tile([128, 64], f32, tag="sc", name=f"sc{rt}")
                nc.vector.tensor_scalar(out=sc[:, :], in0=s8t[:, :],
                                        scalar1=0.0625, scalar2=None, op0=ALU.mult)
                # merged quad combines: {t1,t2} = even+odd, {t3,t4} = even-odd
                # even = {a00,a10}: offsets {0,8}; odd = {a01,a11}: offsets {1,9}
                ev = sc[:, 0:64].rearrange("p (x i y j) -> p i j x y",
                                           x=4, i=2, y=4, j=2)[:, :, 0]
                od = sc[:, 0:64].rearrange("p (x i y j) -> p i j x y",
                                           x=4, i=2, y=4, j=2)[:, :, 1]
                tt = ttp.tile([128, 64], f32, tag="tt", name=f"tt{rt}")
                t2v = lambda o: tt[:, 32 * o:32 * (o + 1)].rearrange(
                    "p (g x y) -> p g x y", g=2, x=4, y=4)
                nc.vector.tensor_add(t2v(0), ev, od)   # t1(a00+a01), t2(a10+a11)
                nc.vector.tensor_sub(t2v(1), ev, od)   # t3, t4
                cf = sp.tile([128, 64], f32, tag="coef", name=f"cf{rt}")
                pick = lambda t, o: t[:, :].rearrange(
                    "p (g h m) -> p h g m", g=2, h=2, m=16)[:, o]
                nc.vector.tensor_add(pick(cf, 0), pick(tt, 0), pick(tt, 1))  # ll, hl
                nc.vector.tensor_sub(pick(cf, 1), pick(tt, 0), pick(tt, 1))  # lh, hh
                coef.append(cf)

                # gate-scaled channel-transposed coefficients: cc[ch][el*64+i, b*64+bm]
                for bh in range(2):
                    b = rt * 2 + bh
                    for e in range(NE):
                        ch, el = e // 2, e % 2
                        nc.vector.tensor_tensor(
                            out=cc[ch][el * 64:(el + 1) * 64, b * 64:(b + 1) * 64],
                            in0=cf[bh * 64:(bh + 1) * 64, :],
                            in1=gt_s[rt][bh * 64:(bh + 1) * 64, 1 + e:2 + e]
                                .broadcast_to([64, 64]),
                            op=ALU.mult,
                        )

            # per-mode channel mixing, gate-combined via K=(e,i) accumulation
            pb = [psp.tile([64, 64], f32, tag="pb", name=f"pb{i}") for i in range(4)]
            for band in range(4):
                for mode in range(16):
                    for ch in range(3):
                        nc.tensor.matmul(
                            out=pb[band][:, mode * 4:(mode + 1) * 4],
                            lhsT=wt_b[band][:, (mode * 3 + ch) * 64:(mode * 3 + ch + 1) * 64],
                            rhs=cc[ch][:, band * 16 + mode:band * 16 + mode + 193:64],
                            start=(ch == 0), stop=(ch == 2),
                        )

            # level-4 Haar synthesis (scale folded into weights) scattered per-sample
            sb1 = ttp.tile([64, 64], f32, tag="sb1")
            sb3 = ttp.tile([64, 64], f32, tag="sb3")
            nc.vector.tensor_copy(sb1[:, :], pb[1][:, :])
            nc.vector.tensor_copy(sb3[:, :], pb[3][:, :])
            u1 = ttp.tile([64, 64], f32, tag="u1")
            u2 = ttp.tile([64, 64], f32, tag="u2")
            u3 = ttp.tile([64, 64], f32, tag="u3")
            u4 = ttp.tile([64, 64], f32, tag="u4")
            nc.vector.tensor_add(u1[:, :], pb[0][:, :], sb1[:, :])
            nc.vector.tensor_add(u2[:, :], pb[2][:, :], sb3[:, :])
            nc.vector.tensor_sub(u3[:, :], pb[0][:, :], sb1[:, :])
            nc.vector.tensor_sub(u4[:, :], pb[2][:, :], sb3[:, :])

            adj_hs = []
            for rt in range(2):
                at = sp.tile([128, 64], f32, tag="adjT", name=f"at{rt}")
                for bh in range(2):
                    b = rt * 2 + bh
                    ov = at[bh * 64:(bh + 1) * 64, :].rearrange(
                        "p (x di y dj) -> p di dj x y", x=4, di=2, y=4, dj=2)
                    sv = lambda t: t[:, :].rearrange("p (x y b) -> p b x y", x=4, y=4, b=4)[:, b]
                    nc.vector.tensor_add(ov[:, 0, 0], sv(u1), sv(u2))
                    nc.vector.tensor_sub(ov[:, 0, 1], sv(u1), sv(u2))
                    nc.vector.tensor_add(ov[:, 1, 0], sv(u3), sv(u4))
                    nc.vector.tensor_sub(ov[:, 1, 1], sv(u3), sv(u4))
                # adjF = adjT + (-G/64) * s8   (gt col 7 = -G/64)
                adjF = sp.tile([128, 64], f32, tag="adjF", name=f"af{rt}")
                nc.vector.scalar_tensor_tensor(
                    out=adjF[:, :], in0=s8[rt][:, :], scalar=gt_s[rt][:, 7:8],
                    in1=at[:, :], op0=ALU.mult, op1=ALU.add,
                )
                # expand over h-rep: adj_h[p, u*64 + dh*8 + v] = adjF[p, u*8+v]
                adj_h = sp.tile([128, 512], f32, tag="adjh", name=f"ah{rt}")
                nc.vector.tensor_copy(
                    out=adj_h[:, :].rearrange("p (u dh v) -> p u dh v", u=8, dh=8, v=8),
                    in_=adjF[:, :].rearrange("p (u o v) -> p u o v", u=8, o=1, v=8)
                        .broadcast_to([128, 8, 8, 8]),
                )
                adj_hs.append(adj_h)

            # y = G*x + rep8(adjF), one fused DVE pass per sub-tile, stores stream out
            for rt in range(2):
                for c in range(NCH):
                    ys = yp.tile([128, 1024], f32, tag="ys", name=f"y{rt}{c}")
                    nc.vector.scalar_tensor_tensor(
                        out=ys[:, :].rearrange("p (hv w) -> p hv w", w=8),
                        in0=xs[rt][c][:, :].rearrange("p (hv w) -> p hv w", w=8),
                        scalar=gt_s[rt][:, 0:1],
                        in1=adj_hs[rt][:, 128 * c:128 * (c + 1)]
                            .rearrange("p (hv o) -> p hv o", o=1)
                            .broadcast_to([128, 128, 8]),
                        op0=ALU.mult, op1=ALU.add,
                    )
                    nc.sync.dma_start(out=yw[rt, :, 1024 * c:1024 * (c + 1)], in_=ys[:, :])
    nc.compile()
    return nc


_NC = None


def _get_nc():
    global _NC
    if _NC is None:
        _NC = _build_nc()
    return _NC


def _pack_weights(WL, WH):
    # Wall[band, e, i, o, x, y]; band 0 = WL, bands 1..3 = WH[:, k-1]
    Wall = np.empty((4, NE, C, C, 4, 4), np.float32)
    Wall[0] = WL[:NE]
    for k in range(3):
        Wall[k + 1] = WH[:NE, k]
    Wall *= 0.0625  # folds idwt 0.5 and rep8 0.125 scales
    # wt[band][el*64+i, ((x*4+y)*3 + ch)*64 + o]
    W6 = Wall.reshape(4, 3, 2, C, C, 4, 4)            # band, ch, el, i, o, x, y
    T = W6.transpose(0, 2, 3, 5, 6, 1, 4)             # band, el, i, x, y, ch, o
    import ml_dtypes
    return np.ascontiguousarray(T.reshape(4, 128, 3072)).astype(ml_dtypes.bfloat16)


def _pack_gates(lambda_):
    lam = lambda_.reshape(B, 8).astype(np.float32)
    G = lam.sum(1)
    geff = lam[:, :6].copy()
    geff[:, 4] += lam[:, 6]
    geff[:, 5] += lam[:, 7]
    gt = np.zeros((B, 8), np.float32)
    gt[:, 0] = G
    gt[:, 1:7] = geff
    gt[:, 7] = -G / 64.0
    return gt


def kernel(x, lambda_, WL, WH):
    from concourse.bass_utils import run_bass_kernel_spmd

    nc = _get_nc()
    wt = _pack_weights(np.asarray(WL, np.float32), np.asarray(WH, np.float32))
    gt = _pack_gates(np.asarray(lambda_, np.float32))
    x = np.ascontiguousarray(np.asarray(x, np.float32))

    in_maps = []
    for k in range(N_CORES):
        xl = x[k * BL:(k + 1) * BL].reshape(2, 128, 4096)
        gl = np.repeat(gt[k * BL:(k + 1) * BL], C, axis=0).reshape(2, 128, 8)
        in_maps.append({"xw": np.ascontiguousarray(xl),
                        "wt": wt,
                        "gt": np.ascontiguousarray(gl)})

    res = run_bass_kernel_spmd(nc, in_maps, list(range(N_CORES)))
    out = np.empty((B, C, S, S), np.float32)
    for k in range(N_CORES):
        out[k * BL:(k + 1) * BL] = res.results[k]["yw"].reshape(BL, C, S, S)
    return out


# revision 14
# speedup vs baseline: 1.1315x; 1.0726x over previous
"""Bass/Trainium2 kernel for nn_Expert_WNO2d (8-expert gated WaveConv2d mixture).

Math: the reference is linear in x. Every expert passes the fine Haar detail
levels (1..3) through unchanged and only channel-mixes the coarsest (level-4)
approximation + detail coefficients. With gate slots s weighting experts
PERM = (0,1,2,3,4,5,4,5), the output collapses to

    y[b] = G[b] * x[b] + rep8( adj[b] )                      (rep8 = 8x8 block broadcast)
    adj[b] = 0.125 * idwt4( sum_e geff[b,e] * (W_e . c4[b]) ) - (G[b]/64) * s8[b]

where s8 = 8x8 block sums of x, c4 = level-4 Haar coefficients (from s8),
G[b] = sum_s lambda[b,s], geff[b,e] = gate mass routed to expert e.

Sharding: data-parallel over batch B=32 across 8 cores (4 samples/core);
the [6,4,C,C,4,4] expert weights are replicated in bf16 (packed host-side
into the exact SBUF image, scaled by 0.0625 to fold the idwt/broadcast
constants). x streams in 1MB sub-tiles with partial block-sum reduces;
weights stream per-band so matmuls start before the full load; the final
fused pass streams per sub-tile (DVE + GpSimd) so y DMA-out overlaps.
"""

import numpy as np

import concourse.bacc as bacc
import concourse.mybir as mybir
import concourse.tile as tile

N_CORES = 8
B, C, S = 32, 64, 64
BL = B // N_CORES          # samples per core = 4
NE = 6                     # live experts
NCH = 4                    # x sub-tiles per row-tile
f32 = mybir.dt.float32
bf16 = mybir.dt.bfloat16
ALU = mybir.AluOpType


def _build_nc():
    nc = bacc.Bacc()
    xw = nc.declare_dram_parameter("xw", [2, 128, 4096], f32, isOutput=False)
    wt = nc.declare_dram_parameter("wt", [4, 128, 3072], bf16, isOutput=False)
    gt = nc.declare_dram_parameter("gt", [2, 128, 8], f32, isOutput=False)
    yw = nc.declare_dram_parameter("yw", [2, 128, 4096], f32, isOutput=True)

    with tile.TileContext(nc) as tc:
        with (
            tc.tile_pool(name="xp", bufs=8) as xp,
            tc.tile_pool(name="yp", bufs=8) as yp,
            tc.tile_pool(name="wp", bufs=4) as wp,
            tc.tile_pool(name="sp", bufs=2) as sp,
            tc.tile_pool(name="cp", bufs=3) as cp,
            tc.tile_pool(name="tp", bufs=8) as ttp,
            tc.tile_pool(name="ps", bufs=4, space="PSUM") as psp,
        ):
            gt_s, xs = [], [[], []]
            for rt in range(2):
                g = sp.tile([128, 8], f32, tag="gt", name=f"g{rt}")
                nc.sync.dma_start(out=g[:, :], in_=gt[rt, :, :])
                gt_s.append(g)

            wt_b = []
            for band in range(4):
                w = wp.tile([128, 3072], bf16, tag="wt", name=f"w{band}")
                wt_b.append(w)

            # interleave x sub-tile and weight-band DMA issue; x leads
            order = [("x", 0, 0), ("x", 0, 1), ("x", 0, 2), ("x", 0, 3),
                     ("x", 1, 0), ("x", 1, 1), ("x", 1, 2), ("x", 1, 3),
                     ("w", 0, 0), ("w", 1, 0), ("w", 2, 0), ("w", 3, 0)]
            for kind, a, c in order:
                if kind == "x":
                    xt = xp.tile([128, 1024], f32, tag="xs", name=f"x{a}{c}")
                    nc.sync.dma_start(out=xt[:, :], in_=xw[a, :, 1024 * c:1024 * (c + 1)])
                    xs[a].append(xt)
                else:
                    nc.sync.dma_start(out=wt_b[a][:, :], in_=wt[a, :, :])

            cc = cp.tile([128, 768], bf16, tag="cc", name="cc")
            coef, s8 = [], []
            for rt in range(2):
                # 8x8 block sums, streamed per sub-tile (w-dir), then h-dir
                r1 = sp.tile([128, 512], f32, tag="r1", name=f"r1{rt}")
                for c in range(NCH):
                    nc.vector.tensor_reduce(
                        out=r1[:, 128 * c:128 * (c + 1)].rearrange("p (h v) -> p h v", h=16),
                        in_=xs[rt][c][:, :].rearrange("p (h v w) -> p h v w", h=16, v=8, w=8),
                        axis=mybir.AxisListType.X, op=ALU.add,
                    )
                s8t = sp.tile([128, 64], f32, tag="s8", name=f"s8{rt}")
                nc.vector.tensor_reduce(
                    out=s8t[:, :].rearrange("p (u v) -> p u v", u=8),
                    in_=r1[:, :].rearrange("p (u dh v) -> p u v dh", u=8, dh=8, v=8),
                    axis=mybir.AxisListType.X, op=ALU.add,
                )
                s8.append(s8t)

                # level-4 Haar analysis on 0.0625*s8 (scale folds ll3 + one dwt level)
                sc = sp.tile([128, 64], f32, tag="sc", name=f"sc{rt}")
                nc.vector.tensor_scalar(out=sc[:, :], in0=s8t[:, :],
                                        scalar1=0.0625, scalar2=None, op0=ALU.mult)
                # merged quad combines: {t1,t2} = even+odd, {t3,t4} = even-odd
                # even = {a00,a10}: offsets {0,8}; odd = {a01,a11}: offsets {1,9}
                ev = sc[:, 0:64].rearrange("p (x i y j) -> p i j x y",
                                           x=4, i=2, y=4, j=2)[:, :, 0]
                od = sc[:, 0:64].rearrange("p (x i y j) -> p i j x y",
                                           x=4, i=2, y=4, j=2)[:, :, 1]
                tt = ttp.tile([128, 64], f32, tag="tt", name=f"tt{rt}")
                t2v = lambda o: tt[:, 32 * o:32 * (o + 1)].rearrange(
                    "p (g x y) -> p g x y", g=2, x=4, y=4)
                nc.vector.tensor_add(t2v(0), ev, od)   # t1(a00+a01), t2(a10+a11)
                nc.vector.tensor_sub(t2v(1), ev, od)   # t3, t4
                cf = sp.tile([128, 64], f32, tag="coef", name=f"cf{rt}")
                pick = lambda t, o: t[:, :].rearrange(
                    "p (g h m) -> p h g m", g=2, h=2, m=16)[:, o]
                nc.vector.tensor_add(pick(cf, 0), pick(tt, 0), pick(tt, 1))  # ll, hl
                nc.vector.tensor_sub(pick(cf, 1), pick(tt, 0), pick(tt, 1))  # lh, hh
                coef.append(cf)

            # gate-scaled channel-transposed coefficients:
            # cc[el*64+i, ch*256 + b*64 + bm], one op per (rt, bh, el):
            # out spans the 3 ch blocks; in0 broadcasts cf over ch; the gate
            # operand walks gt cols 1+el, 3+el, 5+el (stride 2) per ch block.
            for rt in range(2):
                cf = coef[rt]
                for bh in range(2):
                    b = rt * 2 + bh
                    for el in range(2):
                        nc.vector.tensor_tensor(
                            out=cc[el * 64:(el + 1) * 64, :]
                                .rearrange("p (ch bb m) -> p ch bb m", ch=3, bb=4, m=64)[:, :, b],
                            in0=cf[bh * 64:(bh + 1) * 64, :]
                                .rearrange("p (o m) -> p o m", o=1)
                                .broadcast_to([64, 3, 64]),
                            in1=gt_s[rt][bh * 64:(bh + 1) * 64, 1 + el:6 + el:2]
                                .rearrange("p (c o) -> p c o", c=3, o=1)
                                .broadcast_to([64, 3, 64]),
                            op=ALU.mult,
                        )

            # per-mode channel mixing, gate-combined via K=(e,i) accumulation
            pb = [psp.tile([64, 64], f32, tag="pb", name=f"pb{i}") for i in range(4)]
            for band in range(4):
                for mode in range(16):
                    for ch in range(3):
                        nc.tensor.matmul(
                            out=pb[band][:, mode * 4:(mode + 1) * 4],
                            lhsT=wt_b[band][:, (mode * 3 + ch) * 64:(mode * 3 + ch + 1) * 64],
                            rhs=cc[:, ch * 256 + band * 16 + mode:ch * 256 + band * 16 + mode + 193:64],
                            start=(ch == 0), stop=(ch == 2),
                        )

            # level-4 Haar synthesis (scale folded into weights) scattered per-sample
            sb1 = ttp.tile([64, 64], f32, tag="sb1")
            sb3 = ttp.tile([64, 64], f32, tag="sb3")
            nc.vector.tensor_copy(sb1[:, :], pb[1][:, :])
            nc.vector.tensor_copy(sb3[:, :], pb[3][:, :])
            u1 = ttp.tile([64, 64], f32, tag="u1")
            u2 = ttp.tile([64, 64], f32, tag="u2")
            u3 = ttp.tile([64, 64], f32, tag="u3")
            u4 = ttp.tile([64, 64], f32, tag="u4")
            nc.vector.tensor_add(u1[:, :], pb[0][:, :], sb1[:, :])
            nc.vector.tensor_add(u2[:, :], pb[2][:, :], sb3[:, :])
            nc.vector.tensor_sub(u3[:, :], pb[0][:, :], sb1[:, :])
            nc.vector.tensor_sub(u4[:, :], pb[2][:, :], sb3[:, :])

            adj_hs = []
            for rt in range(2):
                at = sp.tile([128, 64], f32, tag="adjT", name=f"at{rt}")
                for bh in range(2):
                    b = rt * 2 + bh
                    ov = at[bh * 64:(bh + 1) * 64, :].rearrange(
                        "p (x di y dj) -> p di dj x y", x=4, di=2, y=4, dj=2)
                    sv = lambda t: t[:, :].rearrange("p (x y b) -> p b x y", x=4, y=4, b=4)[:, b]
                    nc.vector.tensor_add(ov[:, 0, 0], sv(u1), sv(u2))
                    nc.vector.tensor_sub(ov[:, 0, 1], sv(u1), sv(u2))
                    nc.vector.tensor_add(ov[:, 1, 0], sv(u3), sv(u4))
                    nc.vector.tensor_sub(ov[:, 1, 1], sv(u3), sv(u4))
                # adjF = adjT + (-G/64) * s8   (gt col 7 = -G/64)
                adjF = sp.tile([128, 64], f32, tag="adjF", name=f"af{rt}")
                nc.vector.scalar_tensor_tensor(
                    out=adjF[:, :], in0=s8[rt][:, :], scalar=gt_s[rt][:, 7:8],
                    in1=at[:, :], op0=ALU.mult, op1=ALU.add,
                )
                # expand over h-rep: adj_h[p, u*64 + dh*8 + v] = adjF[p, u*8+v]
                adj_h = sp.tile([128, 512], f32, tag="adjh", name=f"ah{rt}")
                nc.vector.tensor_copy(
                    out=adj_h[:, :].rearrange("p (u dh v) -> p u dh v", u=8, dh=8, v=8),
                    in_=adjF[:, :].rearrange("p (u o v) -> p u o v", u=8, o=1, v=8)
                        .broadcast_to([128, 8, 8, 8]),
                )
                adj_hs.append(adj_h)

            # y = G*x + rep8(adjF), one fused DVE pass per sub-tile, stores stream out
            for rt in range(2):
                for c in range(NCH):
                    ys = yp.tile([128, 1024], f32, tag="ys", name=f"y{rt}{c}")
                    nc.vector.scalar_tensor_tensor(
                        out=ys[:, :].rearrange("p (hv w) -> p hv w", w=8),
                        in0=xs[rt][c][:, :].rearrange("p (hv w) -> p hv w", w=8),
                        scalar=gt_s[rt][:, 0:1],
                        in1=adj_hs[rt][:, 128 * c:128 * (c + 1)]
                            .rearrange("p (hv o) -> p hv o", o=1)
                            .broadcast_to([128, 128, 8]),
                        op0=ALU.mult, op1=ALU.add,
                    )
                    nc.sync.dma_start(out=yw[rt, :, 1024 * c:1024 * (c + 1)], in_=ys[:, :])
    nc.compile()
    return nc


_NC = None


def _get_nc():
    global _NC
    if _NC is None:
        _NC = _build_nc()
    return _NC


def _pack_weights(WL, WH):
    # Wall[band, e, i, o, x, y]; band 0 = WL, bands 1..3 = WH[:, k-1]
    Wall = np.empty((4, NE, C, C, 4, 4), np.float32)
    Wall[0] = WL[:NE]
    for k in range(3):
        Wall[k + 1] = WH[:NE, k]
    Wall *= 0.0625  # folds idwt 0.5 and rep8 0.125 scales
    # wt[band][el*64+i, ((x*4+y)*3 + ch)*64 + o]
    W6 = Wall.reshape(4, 3, 2, C, C, 4, 4)            # band, ch, el, i, o, x, y
    T = W6.transpose(0, 2, 3, 5, 6, 1, 4)             # band, el, i, x, y, ch, o
    import ml_dtypes
    return np.ascontiguousarray(T.reshape(4, 128, 3072)).astype(ml_dtypes.bfloat16)


def _pack_gates(lambda_):
    lam = lambda_.reshape(B, 8).astype(np.float32)
    G = lam.sum(1)
    geff = lam[:, :6].copy()
    geff[:, 4] += lam[:, 6]
    geff[:, 5] += lam[:, 7]
    gt = np.zeros((B, 8), np.float32)
    gt[:, 0] = G
    gt[:, 1:7] = geff
    gt[:, 7] = -G / 64.0
    return gt


def kernel(x, lambda_, WL, WH):
    from concourse.bass_utils import run_bass_kernel_spmd

    nc = _get_nc()
    wt = _pack_weights(np.asarray(WL, np.float32), np.asarray(WH, np.float32))
    gt = _pack_gates(np.asarray(lambda_, np.float32))
    x = np.ascontiguousarray(np.asarray(x, np.float32))

    in_maps = []
    for k in range(N_CORES):
        xl = x[k * BL:(k + 1) * BL].reshape(2, 128, 4096)
        gl = np.repeat(gt[k * BL:(k + 1) * BL], C, axis=0).reshape(2, 128, 8)
        in_maps.append({"xw": np.ascontiguousarray(xl),
                        "wt": wt,
                        "gt": np.ascontiguousarray(gl)})

    res = run_bass_kernel_spmd(nc, in_maps, list(range(N_CORES)))
    out = np.empty((B, C, S, S), np.float32)
    for k in range(N_CORES):
        out[k * BL:(k + 1) * BL] = res.results[k]["yw"].reshape(BL, C, S, S)
    return out
